# revision 15
# baseline (speedup 1.0000x reference)
"""Causal multi-head attention with RoPE on 8 Trainium2 NeuronCores.

Problem: B=2, S=2048, D=1024, H=16 heads, head_dim=64, fp32 in/out.

Sharding (hardcoded): 8 cores = 2 (batch) x 4 (head groups of 4 heads).
Core c handles batch b = c // 4 and heads [hg*4, hg*4+4), hg = c % 4.
Each core computes its 4 heads' attention plus the partial output
projection o_part = attn_part @ wo[:, cols].T; the host sums the 4
partials per batch (the row-parallel reduction) to form the output.

Device dataflow per core (all matmuls in bf16, fp32 accumulation):
  qT/kT projections in transposed layout (channels on partitions),
  RoPE applied in that layout: channels of wq/wk are pre-permuted on
  host so each head's dims are [evens, odds]; the pair-rotation then
  needs the half-swapped vector, obtained with a 128x128 permutation
  matmul, and two elementwise multiplies against cos/sin tables.
  Scores are computed transposed, sT = k_rot @ q_rot.T (Sk on
  partitions), exp applied on ScalarE (scale=1/8 folded in), causal
  masking via memset + one triangular-mask multiply on the diagonal
  128-block. A@V uses exp(sT) blocks as the moving operand with
  stationary [v_h | ones] (M=65), so partition 64 of the accumulator
  carries the softmax denominators. Normalization broadcasts 1/r
  across partitions with an accumulating ones-matmul (hi+lo bf16
  split, so the broadcast is fp32-accurate), then the wo projection
  contracts the 256 channels and streams fp32 results to DRAM.
"""

import numpy as np
import ml_dtypes

import concourse.bass as bass
import concourse.mybir as mybir
import concourse.tile as tile_mod
from concourse.bass_utils import run_bass_kernel_spmd

BF16 = ml_dtypes.bfloat16
dt = mybir.dt

B = 2
S = 2048
D = 1024
H = 16
HD = 64          # head dim
HPC = 4          # heads per core
NCH = HPC * HD   # 256 channels per core
KT = D // 128    # 8 contraction tiles over D
NM = S // 128    # 16 seq tiles of 128
NJ = S // 512    # 4 seq chunks of 512
THETA = 10000.0

_CACHE = {}

# Bumped on every kernel change: the Neuron compile cache hashes the HLO
# module WITHOUT the embedded BIR payload, so two different kernels with
# identical I/O signatures collide. A version-sized dummy input forces a
# distinct hash per kernel revision.
KVER = 6


def _split_multi_waits(nc):
    # This container's walrus build rejects >1 sync wait per instruction.
    # Hoist extra waits onto InstEventSemaphore carriers placed before the
    # instruction in the same engine's stream.
    for bb in nc.main_func.blocks:
        new_list = []
        for ins in bb.instructions:
            si = getattr(ins, "sync_info", None)
            if si is not None and si.on_wait and len(si.on_wait) > 1:
                waits = list(si.on_wait)
                si.on_wait = [waits[-1]]
                for w in waits[:-1]:
                    ev = mybir.InstEventSemaphore(
                        name=nc.get_next_instruction_name(),
                        engine=ins.engine,
                        ins=[],
                        outs=[],
                        sync_info=mybir.SyncInfo(on_wait=[w], on_update=[]),
                    )
                    nc.register_instruction(ev, overwrite=True)
                    new_list.append(ev)
            new_list.append(ins)
        bb.instructions[:] = new_list


def _build_nc():
    nc = bass.Bass("TRN2", target_bir_lowering=False)

    # Inputs are shipped in SBUF layout (128 partitions first).
    xT = nc.dram_tensor("xT", [KT * 128, S], dt.bfloat16, kind="ExternalInput")
    wq = nc.dram_tensor("wq", [128, KT * NCH], dt.bfloat16, kind="ExternalInput")
    wk = nc.dram_tensor("wk", [128, KT * NCH], dt.bfloat16, kind="ExternalInput")
    wv = nc.dram_tensor("wv", [128, KT * NCH], dt.bfloat16, kind="ExternalInput")
    wo = nc.dram_tensor("wo", [128, 2 * D], dt.bfloat16, kind="ExternalInput")
    cosd = nc.dram_tensor("cosd", [128, S], dt.float32, kind="ExternalInput")
    sind = nc.dram_tensor("sind", [128, S], dt.float32, kind="ExternalInput")
    perm = nc.dram_tensor("perm", [128, 128], dt.bfloat16, kind="ExternalInput")
    tri = nc.dram_tensor("tri", [128, 128], dt.bfloat16, kind="ExternalInput")
    out = nc.dram_tensor("o", [S, D], dt.float32, kind="ExternalOutput")
    ver = nc.dram_tensor("ver", [1, KVER], dt.float32, kind="ExternalInput")

    EXP = mybir.ActivationFunctionType.Exp

    with tile_mod.TileContext(nc) as tc:
        with (
            tc.tile_pool(name="io", bufs=1) as io,
            tc.tile_pool(name="wk1", bufs=3) as wkp,
            tc.tile_pool(name="ep", bufs=6) as ep,
            tc.tile_pool(name="sm", bufs=2) as sm,
            tc.tile_pool(name="ob", bufs=3) as ob,
            tc.tile_pool(name="ps", bufs=2, space="PSUM") as ps,
            tc.tile_pool(name="pscr", bufs=2, space="PSUM") as scr_p,
            tc.tile_pool(name="po", bufs=2, space="PSUM") as po_p,
        ):
            # xT split per contraction tile and quarter so matmuls can
            # start as data lands; contiguous DRAM slices keep DMA issue cheap
            xTs = []
            for k in range(KT):
                t = io.tile([128, S], dt.bfloat16, tag=f"xT{k}", name=f"xT{k}")
                xTs.append(t)
            for jq in range(NJ):
                for k in range(KT):
                    eng = nc.sync if k % 2 == 0 else nc.gpsimd
                    eng.dma_start(
                        xTs[k][:, jq * 512 : (jq + 1) * 512],
                        xT[k * 128 : (k + 1) * 128, jq * 512 : (jq + 1) * 512],
                    )
            wv_sb = io.tile([128, KT * NCH], dt.bfloat16, tag="wv")
            nc.sync.dma_start(wv_sb[:], wv[:])
            wq_sb = io.tile([128, KT * NCH], dt.bfloat16, tag="wq")
            nc.sync.dma_start(wq_sb[:], wq[:])
            wk_sb = io.tile([128, KT * NCH], dt.bfloat16, tag="wk")
            nc.sync.dma_start(wk_sb[:], wk[:])
            cos_sb = io.tile([128, S], dt.float32, tag="cos")
            nc.sync.dma_start(cos_sb[:], cosd[:])
            sin_sb = io.tile([128, S], dt.float32, tag="sin")
            nc.sync.dma_start(sin_sb[:], sind[:])
            perm_sb = io.tile([128, 128], dt.bfloat16, tag="perm")
            nc.sync.dma_start(perm_sb[:], perm[:])
            tri_sb = io.tile([128, 128], dt.bfloat16, tag="tri")
            nc.sync.dma_start(tri_sb[:], tri[:])
            wo_sb = io.tile([128, 2 * D], dt.bfloat16, tag="wo")
            nc.sync.dma_start(wo_sb[:], wo[:])
            ver_sb = io.tile([1, KVER], dt.float32, tag="ver")
            nc.sync.dma_start(ver_sb[:], ver[:])
            ones_sb = io.tile([1, 64], dt.bfloat16, tag="ones")
            nc.vector.memset(ones_sb[:], 1.0)

            # fine-grained persistent tiles: precise cross-phase dependencies
            q_t = [
                [io.tile([128, 512], dt.bfloat16, tag=f"q{g}{j}", name=f"q{g}{j}")
                 for j in range(NJ)] for g in range(2)
            ]
            k_t = [
                [io.tile([128, 512], dt.bfloat16, tag=f"k{g}{j}", name=f"k{g}{j}")
                 for j in range(NJ)] for g in range(2)
            ]
            v_t = [
                io.tile([128, HPC * 65], dt.bfloat16, tag=f"v{m}", name=f"v{m}")
                for m in range(NM)
            ]
            attn_t = [
                [io.tile([128, 512], dt.bfloat16, tag=f"at{g}{j}", name=f"at{g}{j}")
                 for j in range(NJ)] for g in range(2)
            ]

            def v_proj(m):
                pv = ps.tile([128, NCH], dt.float32, tag="ps", name="pv")
                for k in range(KT):
                    nc.tensor.matmul(
                        pv[:],
                        xTs[k][:, m * 128 : (m + 1) * 128],
                        wv_sb[:, k * NCH : (k + 1) * NCH],
                        start=(k == 0),
                        stop=(k == KT - 1),
                    )
                blk = v_t[m][:].rearrange("p (h c) -> p h c", c=65)
                nc.vector.tensor_copy(
                    blk[:, :, 0:64], pv[:].rearrange("p (h c) -> p h c", c=64)
                )
                nc.gpsimd.memset(blk[:, :, 64:65], 1.0)

            def qk_proj(dst_t, w_sb, g, j):
                pp = ps.tile([128, 512], dt.float32, tag="ps", name="pp")
                for k in range(KT):
                    nc.tensor.matmul(
                        pp[:],
                        w_sb[:, k * NCH + g * 128 : k * NCH + (g + 1) * 128],
                        xTs[k][:, j * 512 : (j + 1) * 512],
                        start=(k == 0),
                        stop=(k == KT - 1),
                    )
                raw = wkp.tile([128, 512], dt.bfloat16, tag="raw")
                nc.scalar.copy(raw[:], pp[:])
                pq = pp
                nc.tensor.matmul(pq[:], perm_sb[:], raw[:], start=True, stop=True)
                t1 = wkp.tile([128, 512], dt.float32, tag="t1")
                nc.gpsimd.tensor_mul(
                    t1[:], raw[:], cos_sb[:, j * 512 : (j + 1) * 512]
                )
                t2 = wkp.tile([128, 512], dt.float32, tag="t2")
                nc.vector.tensor_mul(
                    t2[:], pq[:], sin_sb[:, j * 512 : (j + 1) * 512]
                )
                nc.vector.tensor_add(dst_t[g][j][:], t1[:], t2[:])

            # ---- attention: sT = k_rot @ q_rot.T, exp, A@V with denominators ----
            # Head pairs (2g, 2g+1) interleaved block-by-block: their scores
            # matmuls are K=64 at base partitions 0/64, so the PE runs them
            # concurrently in disjoint row groups. Diagonal blocks narrow all
            # work to the causal column range [128r, 512).
            def attention(hp, j):
                g = hp
                nblk = 4 * j + 4
                pos = [
                    po_p.tile([65, 512], dt.float32, tag="po", name=f"po{t}")
                    for t in range(2)
                ]
                for i in range(nblk):
                    r = i - 4 * j
                    lo = 128 * r if r > 0 else 0
                    psw = scr_p.tile([128, 1024], dt.float32, tag="pscr", name="psw")
                    for t in range(2):
                        off = 64 * t
                        nc.tensor.matmul(
                            psw[:, t * 512 + lo : (t + 1) * 512],
                            k_t[g][i // 4][off : off + 64, (i % 4) * 128 : (i % 4 + 1) * 128],
                            q_t[g][j][off : off + 64, lo:512],
                            start=True,
                            stop=True,
                        )
                    e = ep.tile([128, 1024], dt.bfloat16, tag="e")
                    if lo == 0:
                        nc.scalar.activation(e[:], psw[:], EXP, scale=0.125)
                    else:
                        src_ap = psw[:].rearrange("p (t c) -> p t c", t=2)[:, :, lo:512]
                        dst_ap = e[:].rearrange("p (t c) -> p t c", t=2)[:, :, lo:512]
                        nc.scalar.activation(dst_ap, src_ap, EXP, scale=0.125)
                    if r >= 0:
                        for t in range(2):
                            nc.gpsimd.tensor_mul(
                                e[:, t * 512 + lo : t * 512 + lo + 128],
                                e[:, t * 512 + lo : t * 512 + lo + 128],
                                tri_sb[:],
                            )
                    for t in range(2):
                        h = 2 * hp + t
                        po = pos[t]
                        nc.tensor.matmul(
                            po[0:65, lo:512],
                            v_t[i][:, h * 65 : (h + 1) * 65],
                            e[:, t * 512 + lo : (t + 1) * 512],
                            start=(i == 0),
                            stop=(i == nblk - 1),
                        )
                # normalize: out_h = po[0:64] * broadcast(1 / po[64])
                for t in range(2):
                    off = 64 * t
                    po = pos[t]
                    rec = sm.tile([1, 512], dt.float32, tag="rec")
                    nc.vector.reciprocal(rec[:], po[64:65, :])
                    rhi = sm.tile([1, 512], dt.bfloat16, tag="rhi")
                    nc.vector.tensor_copy(rhi[:], rec[:])
                    rlo = sm.tile([1, 512], dt.bfloat16, tag="rlo")
                    nc.vector.tensor_sub(rlo[:], rec[:], rhi[:])
                    pb = ps.tile([64, 512], dt.float32, tag="ps", name="pb")
                    nc.tensor.matmul(pb[:], ones_sb[:], rhi[:], start=True, stop=False)
                    nc.tensor.matmul(pb[:], ones_sb[:], rlo[:], start=False, stop=True)
                    u_sb = sm.tile([64, 512], dt.float32, tag="u")
                    nc.vector.tensor_copy(u_sb[:], po[0:64, :])
                    nc.vector.tensor_mul(
                        attn_t[g][j][off : off + 64, :], u_sb[:], pb[:]
                    )

            def wo_proj(m, n):
                pf = ps.tile([128, 512], dt.float32, tag="ps", name="pf")
                for g in range(2):
                    nc.tensor.matmul(
                        pf[:],
                        attn_t[g][m // 4][:, (m % 4) * 128 : (m % 4 + 1) * 128],
                        wo_sb[:, g * D + n * 512 : g * D + (n + 1) * 512],
                        start=(g == 0),
                        stop=(g == 1),
                    )
                osb = ob.tile([128, 512], dt.float32, tag="osb")
                nc.vector.tensor_copy(osb[:], pf[:])
                nc.sync.dma_start(
                    out[m * 128 : (m + 1) * 128, n * 512 : (n + 1) * 512], osb[:]
                )

            # ---- emission order sets scheduler priority: overlap phases ----
            order = globals().get("_ORDER", "jmajor")
            if order == "v3":
                for m in range(NM):
                    v_proj(m)
                for j in range(NJ):
                    qk_proj(q_t, wq_sb, 0, j)
                    qk_proj(k_t, wk_sb, 0, j)
                for j in range(NJ):
                    attention(0, j)
                    qk_proj(q_t, wq_sb, 1, j)
                    qk_proj(k_t, wk_sb, 1, j)
                for j in range(NJ):
                    attention(1, j)
                for m in range(NM):
                    wo_proj(m, 0)
                    wo_proj(m, 1)
            elif order == "serial":
                for m in range(NM):
                    v_proj(m)
                for g in range(2):
                    for j in range(NJ):
                        qk_proj(q_t, wq_sb, g, j)
                        qk_proj(k_t, wk_sb, g, j)
                for hp in range(2):
                    for j in range(NJ):
                        attention(hp, j)
                for m in range(NM):
                    wo_proj(m, 0)
                    wo_proj(m, 1)
            elif order == "jmajor":
                for m in range(NM):
                    v_proj(m)
                for j in range(NJ):
                    for g in range(2):
                        qk_proj(q_t, wq_sb, g, j)
                        qk_proj(k_t, wk_sb, g, j)
                for j in (1, 2, 3, 0):
                    attention(0, j)
                    attention(1, j)
                    for mm in range(4 * j, 4 * j + 4):
                        wo_proj(mm, 0)
                        wo_proj(mm, 1)

    _split_multi_waits(nc)
    return nc


def _sbuf_layout(a128xN):
    # (T*128, N) -> (128, T*N) with tile t at columns [t*N, (t+1)*N)
    t = a128xN.shape[0] // 128
    n = a128xN.shape[1]
    return np.ascontiguousarray(
        a128xN.reshape(t, 128, n).transpose(1, 0, 2).reshape(128, t * n)
    )


def _host_prep(x, wq, wk, wv, wo, token_positions):
    x = np.asarray(x, dtype=np.float32)
    wq = np.asarray(wq, dtype=np.float32)
    wk = np.asarray(wk, dtype=np.float32)
    wv = np.asarray(wv, dtype=np.float32)
    wo = np.asarray(wo, dtype=np.float32)
    pos = np.asarray(token_positions).astype(np.float32)

    # deinterleave channel order within each head for q/k: [evens, odds]
    de = np.concatenate([np.arange(0, HD, 2), np.arange(1, HD, 2)])

    # RoPE tables, extended to the 128-partition tile layout
    inv_freq = (1.0 / (THETA ** (np.arange(0, HD, 2, dtype=np.float32) / HD))).astype(
        np.float32
    )
    freqs = pos[:, None] * inv_freq[None, :]  # (S, 32)
    cosT = np.cos(freqs).astype(np.float32).T  # (32, S)
    sinT = np.sin(freqs).astype(np.float32).T
    cos_l = np.ascontiguousarray(np.tile(cosT, (4, 1)))  # (128, S)
    sin_l = np.ascontiguousarray(
        np.concatenate([-sinT, sinT, -sinT, sinT], axis=0)
    )

    # 128x128 half-swap permutation (block diag of two 64-blocks)
    p64 = np.zeros((64, 64), np.float32)
    for i in range(64):
        p64[i, (i + 32) % 64] = 1.0
    perm_l = np.zeros((128, 128), np.float32)
    perm_l[:64, :64] = p64
    perm_l[64:, 64:] = p64

    tri_l = (np.arange(128)[None, :] >= np.arange(128)[:, None]).astype(np.float32)

    in_maps = []
    for c in range(8):
        b, hg = divmod(c, 4)
        rows = hg * NCH + np.arange(NCH)
        # per-head deinterleave for q/k channel rows
        rows_de = (rows.reshape(HPC, HD)[:, de]).reshape(-1)

        xT = np.ascontiguousarray(x[b].T)  # (D, S)
        wq_t = np.ascontiguousarray(wq[rows_de, :].T)  # (D, 256)
        wk_t = np.ascontiguousarray(wk[rows_de, :].T)
        wv_t = np.ascontiguousarray(wv[rows, :].T)
        wo_t = np.ascontiguousarray(wo[:, rows].T)  # (256, D)

        in_maps.append(
            {
                "ver": np.zeros((1, KVER), np.float32),
                "xT": xT.astype(BF16),
                "wq": _sbuf_layout(wq_t).astype(BF16),
                "wk": _sbuf_layout(wk_t).astype(BF16),
                "wv": _sbuf_layout(wv_t).astype(BF16),
                "wo": _sbuf_layout(wo_t).astype(BF16),
                "cosd": cos_l,
                "sind": sin_l,
                "perm": perm_l.astype(BF16),
                "tri": tri_l.astype(BF16),
            }
        )
    return in_maps


def _get_nc():
    if "nc" not in _CACHE:
        _CACHE["nc"] = _build_nc()
    return _CACHE["nc"]


def kernel(x, wq, wk, wv, wo, token_positions, _trace=False, _tmpdir=None):
    nc = _get_nc()
    in_maps = _host_prep(x, wq, wk, wv, wo, token_positions)
    res = run_bass_kernel_spmd(
        nc, in_maps, core_ids=list(range(8)), trace=_trace, tmpdir=_tmpdir
    )
    out = np.zeros((B, S, D), np.float32)
    for c in range(8):
        b = c // 4
        out[b] += res.results[c]["o"]
    if _trace:
        kernel._last_result = res
    return out


# revision 17
# speedup vs baseline: 3.5223x; 3.5223x over previous
"""Causal multi-head attention with RoPE on 8 Trainium2 NeuronCores.

Problem: B=2, S=2048, D=1024, H=16 heads, head_dim=64, fp32 in/out.

Sharding (hardcoded): 8 cores = 2 (batch) x 4 (head groups of 4 heads).
Core c handles batch b = c // 4 and heads [hg*4, hg*4+4), hg = c % 4.
Each core computes its 4 heads' attention plus the partial output
projection o_part = attn_part @ wo[:, cols].T; the host sums the 4
partials per batch (the row-parallel reduction) to form the output.

Device dataflow per core (all matmuls in bf16, fp32 accumulation):
  qT/kT projections in transposed layout (channels on partitions),
  RoPE applied in that layout: channels of wq/wk are pre-permuted on
  host so each head's dims are [evens, odds]; the pair-rotation then
  needs the half-swapped vector, obtained with a 128x128 permutation
  matmul, and two elementwise multiplies against cos/sin tables.
  Scores are computed transposed, sT = k_rot @ q_rot.T (Sk on
  partitions), exp applied on ScalarE (scale=1/8 folded in), causal
  masking via memset + one triangular-mask multiply on the diagonal
  128-block. A@V uses exp(sT) blocks as the moving operand with
  stationary [v_h | ones] (M=65), so partition 64 of the accumulator
  carries the softmax denominators. Normalization broadcasts 1/r
  across partitions with an accumulating ones-matmul (hi+lo bf16
  split, so the broadcast is fp32-accurate), then the wo projection
  contracts the 256 channels and streams fp32 results to DRAM.
"""

import numpy as np
import ml_dtypes

import concourse.bass as bass
import concourse.mybir as mybir
import concourse.tile as tile_mod
from concourse.bass_utils import run_bass_kernel_spmd

BF16 = ml_dtypes.bfloat16
dt = mybir.dt

B = 2
S = 2048
D = 1024
H = 16
HD = 64          # head dim
HPC = 4          # heads per core
NCH = HPC * HD   # 256 channels per core
KT = D // 128    # 8 contraction tiles over D
NM = S // 128    # 16 seq tiles of 128
NJ = S // 512    # 4 seq chunks of 512
THETA = 10000.0

_CACHE = {}

# Bumped on every kernel change: the Neuron compile cache hashes the HLO
# module WITHOUT the embedded BIR payload, so two different kernels with
# identical I/O signatures collide. A version-sized dummy input forces a
# distinct hash per kernel revision.
KVER = 8


def _split_multi_waits(nc):
    # This container's walrus build rejects >1 sync wait per instruction.
    # Hoist extra waits onto InstEventSemaphore carriers placed before the
    # instruction in the same engine's stream.
    for bb in nc.main_func.blocks:
        new_list = []
        for ins in bb.instructions:
            si = getattr(ins, "sync_info", None)
            if si is not None and si.on_wait and len(si.on_wait) > 1:
                waits = list(si.on_wait)
                si.on_wait = [waits[-1]]
                for w in waits[:-1]:
                    ev = mybir.InstEventSemaphore(
                        name=nc.get_next_instruction_name(),
                        engine=ins.engine,
                        ins=[],
                        outs=[],
                        sync_info=mybir.SyncInfo(on_wait=[w], on_update=[]),
                    )
                    nc.register_instruction(ev, overwrite=True)
                    new_list.append(ev)
            new_list.append(ins)
        bb.instructions[:] = new_list


def _build_nc():
    nc = bass.Bass("TRN2", target_bir_lowering=False)

    # Inputs are shipped in SBUF layout (128 partitions first).
    xT = nc.dram_tensor("xT", [KT * 128, S], dt.bfloat16, kind="ExternalInput")
    wq = nc.dram_tensor("wq", [128, KT * NCH], dt.bfloat16, kind="ExternalInput")
    wk = nc.dram_tensor("wk", [128, KT * NCH], dt.bfloat16, kind="ExternalInput")
    wv = nc.dram_tensor("wv", [128, KT * NCH], dt.bfloat16, kind="ExternalInput")
    wo = nc.dram_tensor("wo", [128, 2 * D], dt.bfloat16, kind="ExternalInput")
    cosd = nc.dram_tensor("cosd", [128, S], dt.float32, kind="ExternalInput")
    sind = nc.dram_tensor("sind", [128, S], dt.float32, kind="ExternalInput")
    perm = nc.dram_tensor("perm", [128, 128], dt.bfloat16, kind="ExternalInput")
    tri = nc.dram_tensor("tri", [128, 128], dt.bfloat16, kind="ExternalInput")
    out = nc.dram_tensor("o", [S, D], dt.float32, kind="ExternalOutput")
    ver = nc.dram_tensor("ver", [1, KVER], dt.float32, kind="ExternalInput")

    EXP = mybir.ActivationFunctionType.Exp

    with tile_mod.TileContext(nc) as tc:
        with (
            tc.tile_pool(name="io", bufs=1) as io,
            tc.tile_pool(name="wk1", bufs=3) as wkp,
            tc.tile_pool(name="ep", bufs=6) as ep,
            tc.tile_pool(name="sm", bufs=2) as sm,
            tc.tile_pool(name="ob", bufs=3) as ob,
            tc.tile_pool(name="ps", bufs=2, space="PSUM") as ps,
            tc.tile_pool(name="pscr", bufs=2, space="PSUM") as scr_p,
            tc.tile_pool(name="po", bufs=2, space="PSUM") as po_p,
        ):
            # xT split per contraction tile and quarter so matmuls can
            # start as data lands; contiguous DRAM slices keep DMA issue cheap
            xTs = []
            for k in range(KT):
                t = io.tile([128, S], dt.bfloat16, tag=f"xT{k}", name=f"xT{k}")
                nc.sync.dma_start(t[:], xT[k * 128 : (k + 1) * 128, :])
                xTs.append(t)
            wv_sb = io.tile([128, KT * NCH], dt.bfloat16, tag="wv")
            nc.sync.dma_start(wv_sb[:], wv[:])
            wq_sb = io.tile([128, KT * NCH], dt.bfloat16, tag="wq")
            nc.sync.dma_start(wq_sb[:], wq[:])
            wk_sb = io.tile([128, KT * NCH], dt.bfloat16, tag="wk")
            nc.sync.dma_start(wk_sb[:], wk[:])
            cos_sb = io.tile([128, S], dt.float32, tag="cos")
            nc.sync.dma_start(cos_sb[:], cosd[:])
            sin_sb = io.tile([128, S], dt.float32, tag="sin")
            nc.sync.dma_start(sin_sb[:], sind[:])
            perm_sb = io.tile([128, 128], dt.bfloat16, tag="perm")
            nc.sync.dma_start(perm_sb[:], perm[:])
            tri_sb = io.tile([128, 128], dt.bfloat16, tag="tri")
            nc.sync.dma_start(tri_sb[:], tri[:])
            wo_sb = io.tile([128, 2 * D], dt.bfloat16, tag="wo")
            nc.sync.dma_start(wo_sb[:], wo[:])
            ver_sb = io.tile([1, KVER], dt.float32, tag="ver")
            nc.sync.dma_start(ver_sb[:], ver[:])
            ones_sb = io.tile([1, 64], dt.bfloat16, tag="ones")
            nc.vector.memset(ones_sb[:], 1.0)

            # fine-grained persistent tiles: precise cross-phase dependencies
            q_t = [
                [io.tile([128, 512], dt.bfloat16, tag=f"q{g}{j}", name=f"q{g}{j}")
                 for j in range(NJ)] for g in range(2)
            ]
            k_t = [
                [io.tile([128, 512], dt.bfloat16, tag=f"k{g}{j}", name=f"k{g}{j}")
                 for j in range(NJ)] for g in range(2)
            ]
            v_t = [
                io.tile([128, HPC * 65], dt.bfloat16, tag=f"v{m}", name=f"v{m}")
                for m in range(NM)
            ]
            attn_t = [
                [io.tile([128, 512], dt.bfloat16, tag=f"at{g}{j}", name=f"at{g}{j}")
                 for j in range(NJ)] for g in range(2)
            ]

            def v_proj(m):
                pv = ps.tile([128, NCH], dt.float32, tag="ps", name="pv")
                for k in range(KT):
                    nc.tensor.matmul(
                        pv[:],
                        xTs[k][:, m * 128 : (m + 1) * 128],
                        wv_sb[:, k * NCH : (k + 1) * NCH],
                        start=(k == 0),
                        stop=(k == KT - 1),
                    )
                blk = v_t[m][:].rearrange("p (h c) -> p h c", c=65)
                nc.vector.tensor_copy(
                    blk[:, :, 0:64], pv[:].rearrange("p (h c) -> p h c", c=64)
                )
                nc.vector.memset(blk[:, :, 64:65], 1.0)

            def qk_proj(dst_t, w_sb, g, j):
                pp = ps.tile([128, 512], dt.float32, tag="ps", name="pp")
                for k in range(KT):
                    nc.tensor.matmul(
                        pp[:],
                        w_sb[:, k * NCH + g * 128 : k * NCH + (g + 1) * 128],
                        xTs[k][:, j * 512 : (j + 1) * 512],
                        start=(k == 0),
                        stop=(k == KT - 1),
                    )
                raw = wkp.tile([128, 512], dt.bfloat16, tag="raw")
                nc.scalar.copy(raw[:], pp[:])
                pq = pp
                nc.tensor.matmul(pq[:], perm_sb[:], raw[:], start=True, stop=True)
                t1 = wkp.tile([128, 512], dt.float32, tag="t1")
                nc.vector.tensor_mul(
                    t1[:], raw[:], cos_sb[:, j * 512 : (j + 1) * 512]
                )
                t2 = wkp.tile([128, 512], dt.float32, tag="t2")
                nc.vector.tensor_mul(
                    t2[:], pq[:], sin_sb[:, j * 512 : (j + 1) * 512]
                )
                nc.vector.tensor_add(dst_t[g][j][:], t1[:], t2[:])

            # ---- attention: sT = k_rot @ q_rot.T, exp, A@V with denominators ----
            # Head pairs (2g, 2g+1) interleaved block-by-block: their scores
            # matmuls are K=64 at base partitions 0/64, so the PE runs them
            # concurrently in disjoint row groups. Diagonal blocks narrow all
            # work to the causal column range [128r, 512).
            def attention(hp, j):
                g = hp
                nblk = 4 * j + 4
                pos = [
                    po_p.tile([65, 512], dt.float32, tag="po", name=f"po{t}")
                    for t in range(2)
                ]
                for i in range(nblk):
                    r = i - 4 * j
                    lo = 128 * r if r > 0 else 0
                    psw = scr_p.tile([128, 1024], dt.float32, tag="pscr", name="psw")
                    for t in range(2):
                        off = 64 * t
                        nc.tensor.matmul(
                            psw[:, t * 512 + lo : (t + 1) * 512],
                            k_t[g][i // 4][off : off + 64, (i % 4) * 128 : (i % 4 + 1) * 128],
                            q_t[g][j][off : off + 64, lo:512],
                            start=True,
                            stop=True,
                        )
                    e = ep.tile([128, 1024], dt.bfloat16, tag="e")
                    if lo == 0:
                        nc.scalar.activation(e[:], psw[:], EXP, scale=0.125)
                    else:
                        src_ap = psw[:].rearrange("p (t c) -> p t c", t=2)[:, :, lo:512]
                        dst_ap = e[:].rearrange("p (t c) -> p t c", t=2)[:, :, lo:512]
                        nc.scalar.activation(dst_ap, src_ap, EXP, scale=0.125)
                    if r >= 0:
                        for t in range(2):
                            nc.vector.tensor_mul(
                                e[:, t * 512 + lo : t * 512 + lo + 128],
                                e[:, t * 512 + lo : t * 512 + lo + 128],
                                tri_sb[:],
                            )
                    for t in range(2):
                        h = 2 * hp + t
                        po = pos[t]
                        nc.tensor.matmul(
                            po[0:65, lo:512],
                            v_t[i][:, h * 65 : (h + 1) * 65],
                            e[:, t * 512 + lo : (t + 1) * 512],
                            start=(i == 0),
                            stop=(i == nblk - 1),
                        )
                # normalize: out_h = po[0:64] * broadcast(1 / po[64])
                for t in range(2):
                    off = 64 * t
                    po = pos[t]
                    rec = sm.tile([1, 512], dt.float32, tag="rec")
                    nc.vector.reciprocal(rec[:], po[64:65, :])
                    rhi = sm.tile([1, 512], dt.bfloat16, tag="rhi")
                    nc.vector.tensor_copy(rhi[:], rec[:])
                    rlo = sm.tile([1, 512], dt.bfloat16, tag="rlo")
                    nc.vector.tensor_sub(rlo[:], rec[:], rhi[:])
                    pb = ps.tile([64, 512], dt.float32, tag="ps", name="pb")
                    nc.tensor.matmul(pb[:], ones_sb[:], rhi[:], start=True, stop=False)
                    nc.tensor.matmul(pb[:], ones_sb[:], rlo[:], start=False, stop=True)
                    u_sb = sm.tile([64, 512], dt.float32, tag="u")
                    nc.vector.tensor_copy(u_sb[:], po[0:64, :])
                    nc.vector.tensor_mul(
                        attn_t[g][j][off : off + 64, :], u_sb[:], pb[:]
                    )

            def wo_proj(m, n):
                pf = ps.tile([128, 512], dt.float32, tag="ps", name="pf")
                for g in range(2):
                    nc.tensor.matmul(
                        pf[:],
                        attn_t[g][m // 4][:, (m % 4) * 128 : (m % 4 + 1) * 128],
                        wo_sb[:, g * D + n * 512 : g * D + (n + 1) * 512],
                        start=(g == 0),
                        stop=(g == 1),
                    )
                osb = ob.tile([128, 512], dt.float32, tag="osb")
                nc.vector.tensor_copy(osb[:], pf[:])
                nc.sync.dma_start(
                    out[m * 128 : (m + 1) * 128, n * 512 : (n + 1) * 512], osb[:]
                )

            # ---- emission order sets scheduler priority: overlap phases ----
            order = globals().get("_ORDER", "jmajor")
            if order == "v3":
                for m in range(NM):
                    v_proj(m)
                for j in range(NJ):
                    qk_proj(q_t, wq_sb, 0, j)
                    qk_proj(k_t, wk_sb, 0, j)
                for j in range(NJ):
                    attention(0, j)
                    qk_proj(q_t, wq_sb, 1, j)
                    qk_proj(k_t, wk_sb, 1, j)
                for j in range(NJ):
                    attention(1, j)
                for m in range(NM):
                    wo_proj(m, 0)
                    wo_proj(m, 1)
            elif order == "serial":
                for m in range(NM):
                    v_proj(m)
                for g in range(2):
                    for j in range(NJ):
                        qk_proj(q_t, wq_sb, g, j)
                        qk_proj(k_t, wk_sb, g, j)
                for hp in range(2):
                    for j in range(NJ):
                        attention(hp, j)
                for m in range(NM):
                    wo_proj(m, 0)
                    wo_proj(m, 1)
            elif order == "jmajor":
                for m in range(NM):
                    v_proj(m)
                for j in range(NJ):
                    for g in range(2):
                        qk_proj(q_t, wq_sb, g, j)
                        qk_proj(k_t, wk_sb, g, j)
                for j in (1, 2, 3, 0):
                    attention(0, j)
                    attention(1, j)
                    for mm in range(4 * j, 4 * j + 4):
                        wo_proj(mm, 0)
                        wo_proj(mm, 1)

    _split_multi_waits(nc)
    return nc


def _sbuf_layout(a128xN):
    # (T*128, N) -> (128, T*N) with tile t at columns [t*N, (t+1)*N)
    t = a128xN.shape[0] // 128
    n = a128xN.shape[1]
    return np.ascontiguousarray(
        a128xN.reshape(t, 128, n).transpose(1, 0, 2).reshape(128, t * n)
    )


def _host_prep(x, wq, wk, wv, wo, token_positions):
    x = np.asarray(x, dtype=np.float32)
    wq = np.asarray(wq, dtype=np.float32)
    wk = np.asarray(wk, dtype=np.float32)
    wv = np.asarray(wv, dtype=np.float32)
    wo = np.asarray(wo, dtype=np.float32)
    pos = np.asarray(token_positions).astype(np.float32)

    # deinterleave channel order within each head for q/k: [evens, odds]
    de = np.concatenate([np.arange(0, HD, 2), np.arange(1, HD, 2)])

    # RoPE tables, extended to the 128-partition tile layout
    inv_freq = (1.0 / (THETA ** (np.arange(0, HD, 2, dtype=np.float32) / HD))).astype(
        np.float32
    )
    freqs = pos[:, None] * inv_freq[None, :]  # (S, 32)
    cosT = np.cos(freqs).astype(np.float32).T  # (32, S)
    sinT = np.sin(freqs).astype(np.float32).T
    cos_l = np.ascontiguousarray(np.tile(cosT, (4, 1)))  # (128, S)
    sin_l = np.ascontiguousarray(
        np.concatenate([-sinT, sinT, -sinT, sinT], axis=0)
    )

    # 128x128 half-swap permutation (block diag of two 64-blocks)
    p64 = np.zeros((64, 64), np.float32)
    for i in range(64):
        p64[i, (i + 32) % 64] = 1.0
    perm_l = np.zeros((128, 128), np.float32)
    perm_l[:64, :64] = p64
    perm_l[64:, 64:] = p64

    tri_l = (np.arange(128)[None, :] >= np.arange(128)[:, None]).astype(np.float32)

    in_maps = []
    for c in range(8):
        b, hg = divmod(c, 4)
        rows = hg * NCH + np.arange(NCH)
        # per-head deinterleave for q/k channel rows
        rows_de = (rows.reshape(HPC, HD)[:, de]).reshape(-1)

        xT = np.ascontiguousarray(x[b].T)  # (D, S)
        wq_t = np.ascontiguousarray(wq[rows_de, :].T)  # (D, 256)
        wk_t = np.ascontiguousarray(wk[rows_de, :].T)
        wv_t = np.ascontiguousarray(wv[rows, :].T)
        wo_t = np.ascontiguousarray(wo[:, rows].T)  # (256, D)

        in_maps.append(
            {
                "ver": np.zeros((1, KVER), np.float32),
                "xT": xT.astype(BF16),
                "wq": _sbuf_layout(wq_t).astype(BF16),
                "wk": _sbuf_layout(wk_t).astype(BF16),
                "wv": _sbuf_layout(wv_t).astype(BF16),
                "wo": _sbuf_layout(wo_t).astype(BF16),
                "cosd": cos_l,
                "sind": sin_l,
                "perm": perm_l.astype(BF16),
                "tri": tri_l.astype(BF16),
            }
        )
    return in_maps


def _get_nc():
    if "nc" not in _CACHE:
        _CACHE["nc"] = _build_nc()
    return _CACHE["nc"]


def kernel(x, wq, wk, wv, wo, token_positions, _trace=False, _tmpdir=None):
    nc = _get_nc()
    in_maps = _host_prep(x, wq, wk, wv, wo, token_positions)
    res = run_bass_kernel_spmd(
        nc, in_maps, core_ids=list(range(8)), trace=_trace, tmpdir=_tmpdir
    )
    out = np.zeros((B, S, D), np.float32)
    for c in range(8):
        b = c // 4
        out[b] += res.results[c]["o"]
    if _trace:
        kernel._last_result = res
    return out


# revision 19
# speedup vs baseline: 26.9374x; 7.6477x over previous
"""Causal multi-head attention with RoPE on 8 Trainium2 NeuronCores.

Problem: B=2, S=2048, D=1024, H=16 heads, head_dim=64, fp32 in/out.

Sharding (hardcoded): 8 cores = 2 (batch) x 4 (head groups of 4 heads).
Core c handles batch b = c // 4 and heads [hg*4, hg*4+4), hg = c % 4.
Each core computes its 4 heads' attention plus the partial output
projection o_part = attn_part @ wo[:, cols].T; the host sums the 4
partials per batch (the row-parallel reduction) to form the output.

Device dataflow per core (all matmuls in bf16, fp32 accumulation):
  qT/kT projections in transposed layout (channels on partitions),
  RoPE applied in that layout: channels of wq/wk are pre-permuted on
  host so each head's dims are [evens, odds]; the pair-rotation then
  needs the half-swapped vector, obtained with a 128x128 permutation
  matmul, and two elementwise multiplies against cos/sin tables.
  Scores are computed transposed, sT = k_rot @ q_rot.T (Sk on
  partitions), exp applied on ScalarE (scale=1/8 folded in), causal
  masking via memset + one triangular-mask multiply on the diagonal
  128-block. A@V uses exp(sT) blocks as the moving operand with
  stationary [v_h | ones] (M=65), so partition 64 of the accumulator
  carries the softmax denominators. Normalization broadcasts 1/r
  across partitions with an accumulating ones-matmul (hi+lo bf16
  split, so the broadcast is fp32-accurate), then the wo projection
  contracts the 256 channels and streams fp32 results to DRAM.
"""

import numpy as np
import ml_dtypes

import concourse.bass as bass
import concourse.mybir as mybir
import concourse.tile as tile_mod
from concourse.bass_utils import run_bass_kernel_spmd

BF16 = ml_dtypes.bfloat16
dt = mybir.dt

B = 2
S = 2048
D = 1024
H = 16
HD = 64          # head dim
HPC = 4          # heads per core
NCH = HPC * HD   # 256 channels per core
KT = D // 128    # 8 contraction tiles over D
NM = S // 128    # 16 seq tiles of 128
NJ = S // 512    # 4 seq chunks of 512
THETA = 10000.0

_CACHE = {}

# Bumped on every kernel change: the Neuron compile cache hashes the HLO
# module WITHOUT the embedded BIR payload, so two different kernels with
# identical I/O signatures collide. A version-sized dummy input forces a
# distinct hash per kernel revision.
KVER = 10


def _split_multi_waits(nc):
    # This container's walrus build rejects >1 sync wait per instruction.
    # Hoist extra waits onto InstEventSemaphore carriers placed before the
    # instruction in the same engine's stream.
    for bb in nc.main_func.blocks:
        new_list = []
        for ins in bb.instructions:
            si = getattr(ins, "sync_info", None)
            if si is not None and si.on_wait and len(si.on_wait) > 1:
                waits = list(si.on_wait)
                si.on_wait = [waits[-1]]
                for w in waits[:-1]:
                    ev = mybir.InstEventSemaphore(
                        name=nc.get_next_instruction_name(),
                        engine=ins.engine,
                        ins=[],
                        outs=[],
                        sync_info=mybir.SyncInfo(on_wait=[w], on_update=[]),
                    )
                    nc.register_instruction(ev, overwrite=True)
                    new_list.append(ev)
            new_list.append(ins)
        bb.instructions[:] = new_list


def _build_nc():
    nc = bass.Bass("TRN2", target_bir_lowering=False)

    # Inputs are shipped in SBUF layout (128 partitions first).
    xT = nc.dram_tensor("xT", [KT * 128, S], dt.bfloat16, kind="ExternalInput")
    # packed bf16: wv|wq|wk (3*2048) | perm (128) | tri (128) | wo (2048)
    wpack = nc.dram_tensor("wpack", [128, 8448], dt.bfloat16, kind="ExternalInput")
    # packed f32: cos (2048) | sin (2048) | ver pad (KVER)
    fpack = nc.dram_tensor("fpack", [128, 4096 + KVER], dt.float32, kind="ExternalInput")
    out = nc.dram_tensor("o", [S, D], dt.float32, kind="ExternalOutput")

    EXP = mybir.ActivationFunctionType.Exp

    with tile_mod.TileContext(nc) as tc:
        with (
            tc.tile_pool(name="io", bufs=1) as io,
            tc.tile_pool(name="wk1", bufs=5) as wkp,
            tc.tile_pool(name="ep", bufs=8) as ep,
            tc.tile_pool(name="sm", bufs=4) as sm,
            tc.tile_pool(name="ob", bufs=4) as ob,
            tc.tile_pool(name="ps", bufs=2, space="PSUM") as ps,
            tc.tile_pool(name="pscr", bufs=2, space="PSUM") as scr_p,
            tc.tile_pool(name="po", bufs=2, space="PSUM") as po_p,
        ):
            # xT split per contraction tile and quarter so matmuls can
            # start as data lands; contiguous DRAM slices keep DMA issue cheap
            xTs = []
            for k in range(KT):
                t = io.tile([128, S], dt.bfloat16, tag=f"xT{k}", name=f"xT{k}")
                nc.sync.dma_start(t[:], xT[k * 128 : (k + 1) * 128, :])
                xTs.append(t)
            wpack_sb = io.tile([128, 8448], dt.bfloat16, tag="wpack")
            nc.sync.dma_start(wpack_sb[:], wpack[:])
            fpack_sb = io.tile([128, 4096 + KVER], dt.float32, tag="fpack")
            nc.sync.dma_start(fpack_sb[:], fpack[:])
            wv_sb = wpack_sb[:, 0:2048]
            wq_sb = wpack_sb[:, 2048:4096]
            wk_sb = wpack_sb[:, 4096:6144]
            perm_sb = wpack_sb[:, 6144:6272]
            tri_sb = wpack_sb[:, 6272:6400]
            wo_sb = wpack_sb[:, 6400:8448]
            cos_sb = fpack_sb[:, 0:2048]
            sin_sb = fpack_sb[:, 2048:4096]
            ones_sb = io.tile([1, 64], dt.bfloat16, tag="ones")
            nc.vector.memset(ones_sb[:], 1.0)

            # fine-grained persistent tiles: precise cross-phase dependencies
            q_t = [
                [io.tile([128, 512], dt.bfloat16, tag=f"q{g}{j}", name=f"q{g}{j}")
                 for j in range(NJ)] for g in range(2)
            ]
            k_t = [
                [io.tile([128, 512], dt.bfloat16, tag=f"k{g}{j}", name=f"k{g}{j}")
                 for j in range(NJ)] for g in range(2)
            ]
            v_t = [
                io.tile([128, HPC * 65], dt.bfloat16, tag=f"v{m}", name=f"v{m}")
                for m in range(NM)
            ]
            attn_t = [
                [io.tile([128, 512], dt.bfloat16, tag=f"at{g}{j}", name=f"at{g}{j}")
                 for j in range(NJ)] for g in range(2)
            ]

            def v_proj(m):
                pv = ps.tile([128, NCH], dt.float32, tag="ps", name="pv")
                for k in range(KT):
                    nc.tensor.matmul(
                        pv[:],
                        xTs[k][:, m * 128 : (m + 1) * 128],
                        wv_sb[:, k * NCH : (k + 1) * NCH],
                        start=(k == 0),
                        stop=(k == KT - 1),
                    )
                blk = v_t[m][:].rearrange("p (h c) -> p h c", c=65)
                nc.vector.tensor_copy(
                    blk[:, :, 0:64], pv[:].rearrange("p (h c) -> p h c", c=64)
                )
                nc.vector.memset(blk[:, :, 64:65], 1.0)

            def qk_proj(dst_t, w_sb, g, j):
                pp = ps.tile([128, 512], dt.float32, tag="ps", name="pp")
                for k in range(KT):
                    nc.tensor.matmul(
                        pp[:],
                        w_sb[:, k * NCH + g * 128 : k * NCH + (g + 1) * 128],
                        xTs[k][:, j * 512 : (j + 1) * 512],
                        start=(k == 0),
                        stop=(k == KT - 1),
                    )
                raw = wkp.tile([128, 512], dt.bfloat16, tag="raw")
                nc.scalar.copy(raw[:], pp[:])
                pq = pp
                nc.tensor.matmul(pq[:], perm_sb, raw[:], start=True, stop=True)
                t1 = wkp.tile([128, 512], dt.float32, tag="t1")
                nc.vector.tensor_mul(
                    t1[:], raw[:], cos_sb[:, j * 512 : (j + 1) * 512]
                )
                t2 = wkp.tile([128, 512], dt.float32, tag="t2")
                nc.vector.tensor_mul(
                    t2[:], pq[:], sin_sb[:, j * 512 : (j + 1) * 512]
                )
                nc.vector.tensor_add(dst_t[g][j][:], t1[:], t2[:])

            # ---- attention: sT = k_rot @ q_rot.T, exp, A@V with denominators ----
            # Head pairs (2g, 2g+1) interleaved block-by-block: their scores
            # matmuls are K=64 at base partitions 0/64, so the PE runs them
            # concurrently in disjoint row groups. Diagonal blocks narrow all
            # work to the causal column range [128r, 512).
            def attention(hp, j):
                g = hp
                nblk = 4 * j + 4
                pos = [
                    po_p.tile([65, 512], dt.float32, tag="po", name=f"po{t}")
                    for t in range(2)
                ]
                for i in range(nblk):
                    r = i - 4 * j
                    lo = 128 * r if r > 0 else 0
                    psw = scr_p.tile([128, 1024], dt.float32, tag="pscr", name="psw")
                    for t in range(2):
                        off = 64 * t
                        nc.tensor.matmul(
                            psw[:, t * 512 + lo : (t + 1) * 512],
                            k_t[g][i // 4][off : off + 64, (i % 4) * 128 : (i % 4 + 1) * 128],
                            q_t[g][j][off : off + 64, lo:512],
                            start=True,
                            stop=True,
                        )
                    e = ep.tile([128, 1024], dt.bfloat16, tag="e")
                    if lo == 0:
                        nc.scalar.activation(e[:], psw[:], EXP, scale=0.125)
                    else:
                        src_ap = psw[:].rearrange("p (t c) -> p t c", t=2)[:, :, lo:512]
                        dst_ap = e[:].rearrange("p (t c) -> p t c", t=2)[:, :, lo:512]
                        nc.scalar.activation(dst_ap, src_ap, EXP, scale=0.125)
                    if r >= 0:
                        for t in range(2):
                            nc.vector.tensor_mul(
                                e[:, t * 512 + lo : t * 512 + lo + 128],
                                e[:, t * 512 + lo : t * 512 + lo + 128],
                                tri_sb,
                            )
                    for t in range(2):
                        h = 2 * hp + t
                        po = pos[t]
                        nc.tensor.matmul(
                            po[0:65, lo:512],
                            v_t[i][:, h * 65 : (h + 1) * 65],
                            e[:, t * 512 + lo : (t + 1) * 512],
                            start=(i == 0),
                            stop=(i == nblk - 1),
                        )
                # normalize: out_h = po[0:64] * broadcast(1 / po[64])
                for t in range(2):
                    off = 64 * t
                    po = pos[t]
                    rec = sm.tile([1, 512], dt.float32, tag="rec")
                    nc.vector.reciprocal(rec[:], po[64:65, :])
                    rhi = sm.tile([1, 512], dt.bfloat16, tag="rhi")
                    nc.vector.tensor_copy(rhi[:], rec[:])
                    rlo = sm.tile([1, 512], dt.bfloat16, tag="rlo")
                    nc.vector.tensor_sub(rlo[:], rec[:], rhi[:])
                    pb = ps.tile([64, 512], dt.float32, tag="ps", name="pb")
                    nc.tensor.matmul(pb[:], ones_sb[:], rhi[:], start=True, stop=False)
                    nc.tensor.matmul(pb[:], ones_sb[:], rlo[:], start=False, stop=True)
                    u_sb = sm.tile([64, 512], dt.float32, tag="u")
                    nc.vector.tensor_copy(u_sb[:], po[0:64, :])
                    nc.vector.tensor_mul(
                        attn_t[g][j][off : off + 64, :], u_sb[:], pb[:]
                    )

            def wo_proj(m, n):
                pf = ps.tile([128, 512], dt.float32, tag="ps", name="pf")
                for g in range(2):
                    nc.tensor.matmul(
                        pf[:],
                        attn_t[g][m // 4][:, (m % 4) * 128 : (m % 4 + 1) * 128],
                        wo_sb[:, g * D + n * 512 : g * D + (n + 1) * 512],
                        start=(g == 0),
                        stop=(g == 1),
                    )
                osb = ob.tile([128, 512], dt.float32, tag="osb")
                if n == 0:
                    nc.vector.tensor_copy(osb[:], pf[:])
                else:
                    nc.scalar.copy(osb[:], pf[:])
                nc.sync.dma_start(
                    out[m * 128 : (m + 1) * 128, n * 512 : (n + 1) * 512], osb[:]
                )

            # ---- emission order sets scheduler priority: overlap phases ----
            order = globals().get("_ORDER", "jmajor")
            if order == "v3":
                for m in range(NM):
                    v_proj(m)
                for j in range(NJ):
                    qk_proj(q_t, wq_sb, 0, j)
                    qk_proj(k_t, wk_sb, 0, j)
                for j in range(NJ):
                    attention(0, j)
                    qk_proj(q_t, wq_sb, 1, j)
                    qk_proj(k_t, wk_sb, 1, j)
                for j in range(NJ):
                    attention(1, j)
                for m in range(NM):
                    wo_proj(m, 0)
                    wo_proj(m, 1)
            elif order == "serial":
                for m in range(NM):
                    v_proj(m)
                for g in range(2):
                    for j in range(NJ):
                        qk_proj(q_t, wq_sb, g, j)
                        qk_proj(k_t, wk_sb, g, j)
                for hp in range(2):
                    for j in range(NJ):
                        attention(hp, j)
                for m in range(NM):
                    wo_proj(m, 0)
                    wo_proj(m, 1)
            elif order == "jmajor":
                for m in range(NM):
                    v_proj(m)
                for j in range(NJ):
                    for g in range(2):
                        qk_proj(q_t, wq_sb, g, j)
                        qk_proj(k_t, wk_sb, g, j)
                for j in (1, 2, 3, 0):
                    attention(0, j)
                    attention(1, j)
                    for mm in range(4 * j, 4 * j + 4):
                        wo_proj(mm, 0)
                        wo_proj(mm, 1)

    _split_multi_waits(nc)
    return nc


def _sbuf_layout(a128xN):
    # (T*128, N) -> (128, T*N) with tile t at columns [t*N, (t+1)*N)
    t = a128xN.shape[0] // 128
    n = a128xN.shape[1]
    return np.ascontiguousarray(
        a128xN.reshape(t, 128, n).transpose(1, 0, 2).reshape(128, t * n)
    )


def _host_prep(x, wq, wk, wv, wo, token_positions):
    x = np.asarray(x, dtype=np.float32)
    wq = np.asarray(wq, dtype=np.float32)
    wk = np.asarray(wk, dtype=np.float32)
    wv = np.asarray(wv, dtype=np.float32)
    wo = np.asarray(wo, dtype=np.float32)
    pos = np.asarray(token_positions).astype(np.float32)

    # deinterleave channel order within each head for q/k: [evens, odds]
    de = np.concatenate([np.arange(0, HD, 2), np.arange(1, HD, 2)])

    # RoPE tables, extended to the 128-partition tile layout
    inv_freq = (1.0 / (THETA ** (np.arange(0, HD, 2, dtype=np.float32) / HD))).astype(
        np.float32
    )
    freqs = pos[:, None] * inv_freq[None, :]  # (S, 32)
    cosT = np.cos(freqs).astype(np.float32).T  # (32, S)
    sinT = np.sin(freqs).astype(np.float32).T
    cos_l = np.ascontiguousarray(np.tile(cosT, (4, 1)))  # (128, S)
    sin_l = np.ascontiguousarray(
        np.concatenate([-sinT, sinT, -sinT, sinT], axis=0)
    )

    # 128x128 half-swap permutation (block diag of two 64-blocks)
    p64 = np.zeros((64, 64), np.float32)
    for i in range(64):
        p64[i, (i + 32) % 64] = 1.0
    perm_l = np.zeros((128, 128), np.float32)
    perm_l[:64, :64] = p64
    perm_l[64:, 64:] = p64

    tri_l = (np.arange(128)[None, :] >= np.arange(128)[:, None]).astype(np.float32)

    in_maps = []
    for c in range(8):
        b, hg = divmod(c, 4)
        rows = hg * NCH + np.arange(NCH)
        # per-head deinterleave for q/k channel rows
        rows_de = (rows.reshape(HPC, HD)[:, de]).reshape(-1)

        xT = np.ascontiguousarray(x[b].T)  # (D, S)
        wq_t = np.ascontiguousarray(wq[rows_de, :].T)  # (D, 256)
        wk_t = np.ascontiguousarray(wk[rows_de, :].T)
        wv_t = np.ascontiguousarray(wv[rows, :].T)
        wo_t = np.ascontiguousarray(wo[:, rows].T)  # (256, D)

        wpk = np.concatenate(
            [
                _sbuf_layout(wv_t),
                _sbuf_layout(wq_t),
                _sbuf_layout(wk_t),
                perm_l,
                tri_l,
                _sbuf_layout(wo_t),
            ],
            axis=1,
        ).astype(BF16)
        fpk = np.concatenate(
            [cos_l, sin_l, np.zeros((128, KVER), np.float32)], axis=1
        ).astype(np.float32)
        in_maps.append({"xT": xT.astype(BF16), "wpack": wpk, "fpack": fpk})
    return in_maps


def _get_nc():
    if "nc" not in _CACHE:
        _CACHE["nc"] = _build_nc()
    return _CACHE["nc"]


def kernel(x, wq, wk, wv, wo, token_positions, _trace=False, _tmpdir=None):
    nc = _get_nc()
    in_maps = _host_prep(x, wq, wk, wv, wo, token_positions)
    res = run_bass_kernel_spmd(
        nc, in_maps, core_ids=list(range(8)), trace=_trace, tmpdir=_tmpdir
    )
    out = np.zeros((B, S, D), np.float32)
    for c in range(8):
        b = c // 4
        out[b] += res.results[c]["o"]
    if _trace:
        kernel._last_result = res
    return out
